# revision 57
# baseline (speedup 1.0000x reference)
"""Multi-head attention (B=2, S=2048, EMB=1024, 16 heads) on 8 Trainium2 cores.

Sharding: core c -> batch c//4, head-group c%4 (4 heads = 256 projection dims).
Each core computes Q/K projections in transposed layout (head-dim on
partitions), V natural, attention without max subtraction (scores ~ N(0,1) in
fp32), the softmax denominator via a ones-column appended to V (free inside
the ctx matmul M=65), and a row-parallel partial of the output projection.
The host sums the 4 partials per batch and adds the output bias.

v2 design (driven by the TimelineSim cost model, which tracks HW within ~6%):
  - matmul cost = out free-size x cycles/col, independent of K -> ctx matmuls
    use single K=128 chains (16 kt steps) instead of K=64 half-chains,
    halving ctx PE time. Consecutive mms alternate PSUM banks (hi index) so
    LDWEIGHTS pipelines on HW.
  - exp runs on [128, 1024] tiles (both heads of a pair share one 2-bank
    PSUM score tile) halving the per-instruction ACT access overhead.
  - all SBUF/DRAM tensors are bf16 (PSUM stays fp32): same PE rate, half
    the DMA traffic (phase 1 was DMA-bound), half the SBUF footprint.
  - x inputs arrive as one [128, 4096] slab DMA per (tensor, chunk) spread
    over the SP/ACT/DVE queue rings (96 small DMAs -> 12 big ones).
  - softmax 1/den broadcast: one K=2 matmul per head-pair fans both heads'
    reciprocal rows across 128 partitions (sel2 one-hot).
  - normalization is deferred by one head-pair, out-projection by one query
    chunk; both are sliced into the ACT-bound attention loop (one slice per
    kt step) where PE/DVE have slack.
PSUM: spoolA/B 1x[128,1024] each (scores / projections / bcast rotating,
4 banks) + ctx pool 4x[128,512] (ctx chains + out-proj, 4 banks) = 8 banks.
"""

import numpy as np

import concourse.tile as tile
from concourse import bacc, mybir
from concourse import bass_utils

EMB = 1024
S = 2048
B = 2
HPC = 4            # heads per core
DQ = HPC * 64      # 256 projection dims per core
NCORES = 8

F32 = mybir.dt.float32
BF16 = mybir.dt.bfloat16
EXP = mybir.ActivationFunctionType.Exp

KT_E = EMB // 128  # 8 contraction tiles over EMB
NQC = S // 512     # 4 query chunks
NST = S // 128     # 16 sequence tiles

NP_BF16 = mybir.dt.np(BF16)

_NC = None
TRACE = False
LAST_RESULT = None
STAGE = "full"   # "full" | "nofin" (skip norm/outproj/qproj) | "noodma" (skip out DMA) | "noxdma" (reuse one slab)


def _mha(ctx, tc, xqT, xkT, xvT, wqT, wkT, wvT, woT, bq, bk, bv, out,
         nrm_scr=None, bench_iters=None):
    nc = tc.nc

    cstp = ctx.enter_context(tc.tile_pool(name="const", bufs=1))
    xpool = ctx.enter_context(tc.tile_pool(name="xin", bufs=6))
    epool = ctx.enter_context(tc.tile_pool(name="exp", bufs=4))
    opool = ctx.enter_context(tc.tile_pool(name="osb", bufs=3))
    spoolA = ctx.enter_context(tc.tile_pool(name="scpsA", bufs=1, space="PSUM"))
    spoolB = ctx.enter_context(tc.tile_pool(name="scpsB", bufs=1, space="PSUM"))
    cpool = ctx.enter_context(tc.tile_pool(name="ctxps", bufs=4, space="PSUM"))
    bspool = ctx.enter_context(tc.tile_pool(name="bsnorm", bufs=2))

    # ---- persistent SBUF tensors ----
    ones_row = cstp.tile([1, 512], BF16)
    # 1/den rows live at partitions 0 and 64 (legal engine starts); they are
    # broadcast across partitions via a DRAM round-trip DMA (stride-0 source),
    # keeping the whole softmax normalization off the PE instruction queue.
    rdens = [cstp.tile([65, 512], BF16, name=f"rden{i}") for i in range(2)]
    with nc.allow_low_precision(reason="bf16 constants"):
        nc.vector.memset(ones_row[:], 1.0)

    wq_sb = cstp.tile([128, KT_E * DQ], BF16)   # [128, 2048]: wq_sb[p, n*256+m] = WqT[n*128+p, m]
    wk_sb = cstp.tile([128, KT_E * DQ], BF16)
    wv_sb = cstp.tile([128, KT_E * DQ], BF16)
    wo_sb = cstp.tile([128, 2 * EMB], BF16)     # wo_sb[p, n*1024+f] = WoT[n*128+p, f]
    # per-partition bias columns: b?c[p, dq] = bias[dq*128+p]
    bqc = cstp.tile([128, 2], F32)
    bkc = cstp.tile([128, 2], F32)
    bv_sb = cstp.tile([1, DQ], BF16)

    preslabs = {}
    if STAGE == "noxdma":
        for nm, src, qc in ([(f"xk_{i}", xkT, i) for i in range(NQC)]
                            + [(f"xv_{i}", xvT, i) for i in range(NQC)]
                            + [(f"xq_{i}", xqT, i) for i in range(NQC)]):
            t = cstp.tile([128, KT_E * 512], BF16, name=f"pre_{nm}")
            nc.sync.dma_start(t[:], src[qc])
            preslabs[nm] = t

    # results of phase 1 kept resident
    kT_sb = cstp.tile([128, 2 * S], BF16)       # [dq-block 2][s 2048]
    qT_sb = cstp.tile([128, 2 * S], BF16)
    ctxT_sb = cstp.tile([128, 2 * S], BF16)
    v_sb = cstp.tile([128, NST * (HPC * 65)], BF16)  # per s-tile: 4 heads x (64 V + ones col)
    with nc.allow_low_precision(reason="init"):
        nc.vector.memset(ctxT_sb[:], 0.001)
        nc.vector.memset(qT_sb[:], 0.001)
    with nc.allow_low_precision(reason="bf16 ones column"):
        nc.vector.memset(
            v_sb[:].rearrange("p (t h m) -> p t h m", t=NST, h=HPC)[:, :, :, 64:65],
            1.0,
        )

    # ---- weight / bias DMAs; wk leads the SP ring so the first projection
    # starts ASAP. Slow SWDGE (gpsimd) ring only carries non-urgent copies. ----
    nc.sync.dma_start(
        wk_sb[:].rearrange("p (n m) -> p n m", n=KT_E),
        wkT.rearrange("(n p) m -> p n m", p=128),
    )
    nc.scalar.dma_start(
        wv_sb[:].rearrange("p (n m) -> p n m", n=KT_E),
        wvT.rearrange("(n p) m -> p n m", p=128),
    )
    nc.gpsimd.dma_start(bkc[:], bk.rearrange("o (d p) -> p (o d)", p=128))
    nc.gpsimd.dma_start(bqc[:], bq.rearrange("o (d p) -> p (o d)", p=128))
    nc.gpsimd.dma_start(bv_sb[:], bv[:])
    nc.gpsimd.dma_start(
        wq_sb[:].rearrange("p (n m) -> p n m", n=KT_E),
        wqT.rearrange("(n p) m -> p n m", p=128),
    )
    nc.gpsimd.dma_start(
        wo_sb[:].rearrange("p (n m) -> p n m", n=2),
        woT.rearrange("(n p) m -> p n m", p=128),
    )

    def body():
        _body(tc, nc, xqT, xkT, xvT, out, ones_row, nrm_scr, rdens, wq_sb, wk_sb,
              wv_sb, wo_sb, bqc, bkc, bv_sb, kT_sb, qT_sb, ctxT_sb, v_sb,
              xpool, epool, opool, spoolA, spoolB, cpool, bspool, preslabs)

    if bench_iters:
        hints = (
            mybir.EngineType.PE,
            mybir.EngineType.Activation,
            mybir.EngineType.DVE,
            mybir.EngineType.SP,
            mybir.EngineType.Pool,
        )
        with tc.For_i(0, bench_iters, 1, hint_engines=hints):
            body()
    else:
        body()


def _body(tc, nc, xqT, xkT, xvT, out, ones_row, nrm_scr, rdens, wq_sb, wk_sb,
          wv_sb, wo_sb, bqc, bkc, bv_sb, kT_sb, qT_sb, ctxT_sb, v_sb,
          xpool, epool, opool, spoolA, spoolB, cpool, bspool, preslabs):
    fin = []  # FIFO of deferred finish slices, popped one per kt step

    def pop():
        if fin:
            fin.pop(0)()

    do_norm = STAGE not in ("nofin", "nonorm", "on_out", "on_qp")
    do_out = STAGE not in ("nofin", "noout", "on_norm", "on_qp", "noodma2")
    do_qp = STAGE not in ("nofin", "noqp", "on_norm", "on_out", "qpseam")

    def queue(idx, sl):
        if do_norm:
            fin.insert(idx, sl) if idx is not None else fin.append(sl)

    spools = [spoolA, spoolB]
    sp_state = [0]

    def next_spool(name):
        t = spools[sp_state[0] % 2].tile([128, 1024], F32, tag="sc", name=name)
        sp_state[0] += 1
        return t

    def slab_dma(eng, src, qc, name):
        if STAGE == "noxdma":
            return preslabs[name]
        t = xpool.tile([128, KT_E * 512], BF16, tag="slab", name=name)
        eng.dma_start(t[:], src[qc])
        return t

    def proj_mms(ps, qc, w_sb, xs, kts):
        # two dq chains into the two banks of one [128,1024] psum tile
        for kt in kts:
            for dq in range(2):
                nc.tensor.matmul(
                    ps[:, dq * 512: dq * 512 + 512],
                    w_sb[:, kt * DQ + dq * 128: kt * DQ + dq * 128 + 128],
                    xs[:, kt * 512: kt * 512 + 512],
                    start=(kt == 0), stop=(kt == KT_E - 1),
                )

    def proj_drain(ps, dst_sb, qc, bcol):
        for dq in range(2):
            nc.vector.tensor_scalar_add(
                dst_sb[:, dq * S + qc * 512: dq * S + qc * 512 + 512],
                ps[:, dq * 512: dq * 512 + 512], bcol[:, dq: dq + 1],
            )

    def proj_qk(qc, w_sb, xs, dst_sb, bcol):
        ps = next_spool(f"p_{qc}")
        proj_mms(ps, qc, w_sb, xs, range(KT_E))
        proj_drain(ps, dst_sb, qc, bcol)

    def proj_v(qc, xv):
        for pair in range(2):
            vt = next_spool(f"vps_{qc}_{pair}")
            for kt in range(KT_E):
                for i in range(2):
                    sti = pair * 2 + i
                    nc.tensor.matmul(
                        vt[:, i * 512: i * 512 + DQ],
                        xv[:, kt * 512 + sti * 128: kt * 512 + sti * 128 + 128],
                        wv_sb[:, kt * DQ: kt * DQ + DQ],
                        start=(kt == 0), stop=False,
                    )
            for i in range(2):
                nc.tensor.matmul(
                    vt[:, i * 512: i * 512 + DQ], ones_row[0:1, 0:128], bv_sb[0:1, :],
                    start=False, stop=True,
                )
            for i in range(2):
                st = qc * 4 + pair * 2 + i
                dst = v_sb[:, st * (HPC * 65): (st + 1) * (HPC * 65)]
                nc.vector.tensor_copy(
                    dst.rearrange("p (h m) -> p h m", h=HPC)[:, :, 0:64],
                    vt[:, i * 512: i * 512 + DQ].rearrange("p (h m) -> p h m", h=HPC),
                )

    # ---- finish slices -------------------------------------------------
    # Normalization of (qc, hp): 1/den rows (DVE) -> PE broadcast of each
    # head's 1/den into the spare rows 64:128 of the *other* head's ctx tile
    # (no extra PSUM bank) -> per-head multiply into ctxT_sb.
    def ctx_tail_slice(ctx_mms, prev, hp, cps):
        def go():
            ctx_mms(*prev)
            for hi in range(2):
                with nc.allow_low_precision(reason="bf16 reciprocal of softmax denom"):
                    nc.vector.reciprocal(
                        rdens[hp][64 * hi: 64 * hi + 1, :], cps[hi][64:65, :]
                    )
        return go

    from concourse.bass import AP

    def rden_out_slice(qc, hp):
        def go():
            w = qc * 2 + hp
            r = rdens[hp][:]
            src_ap = AP(tensor=r.tensor, offset=r.offset, ap=[[64 * 512, 2], [1, 512]])
            nc.gpsimd.dma_start(nrm_scr[w], src_ap)
        return go

    def bs_in_slice(qc, hp, hi, st):
        def go():
            w = qc * 2 + hp
            if hi == 0:
                st.append(bspool.tile([128, 512], BF16, tag="bs", name=f"bs_{qc}_{hp}"))
            bs = st[0]
            s = nrm_scr[w, hi, :]
            src_ap = AP(tensor=s.tensor, offset=s.offset, ap=[[0, 64], [1, 512]])
            nc.gpsimd.dma_start(bs[64 * hi: 64 * hi + 64, :], src_ap)
        return go

    def mul_slice(qc, hp, hi, cps, st):
        def go():
            bs = st[0]
            nc.vector.tensor_mul(
                ctxT_sb[64 * hi: 64 * hi + 64, hp * S + qc * 512: hp * S + qc * 512 + 512],
                cps[hi][0:64, :],
                bs[64 * hi: 64 * hi + 64, :],
            )
        return go

    def qproj_mm_slice(ps_box, qc1, xs, step):
        def go():
            if step == 0:
                ps_box.append(
                    [cpool.tile([128, 512], F32, tag="ctx", name=f"qp_{qc1}_{dq}")
                     for dq in range(2)]
            )
            psd = ps_box[0]
            for dq in range(2):
                nc.tensor.matmul(
                    psd[dq][:],
                    wq_sb[:, step * DQ + dq * 128: step * DQ + dq * 128 + 128],
                    xs[:, step * 512: step * 512 + 512],
                    start=(step == 0), stop=(step == KT_E - 1),
                )
        return go

    def qproj_tail_slice(ps_box, qc1):
        def go():
            psd = ps_box[0]
            for dq in range(2):
                nc.vector.tensor_scalar_add(
                    qT_sb[:, dq * S + qc1 * 512: dq * S + qc1 * 512 + 512],
                    psd[dq][:], bqc[:, dq: dq + 1],
                )
        return go

    def queue_qproj(qc1, xs):
        ps_box = []
        for step in range(KT_E):
            fin.append(qproj_mm_slice(ps_box, qc1, xs, step))
        fin.append(qproj_tail_slice(ps_box, qc1))

    def out_mm_slice(qt, st):
        def go():
            ps = [cpool.tile([128, 512], F32, tag="ctx", name=f"ops_{qt}_{fc}")
                  for fc in range(2)]
            for dq in range(2):
                for fc in range(2):
                    nc.tensor.matmul(
                        ps[fc][:],
                        ctxT_sb[:, dq * S + qt * 128: dq * S + qt * 128 + 128],
                        wo_sb[:, dq * EMB + fc * 512: dq * EMB + fc * 512 + 512],
                        start=(dq == 0), stop=(dq == 1),
                    )
            st.append(ps)
        return go

    def out_drain_slice(qt, st):
        def go():
            ps = st.pop(0)
            ot = opool.tile([128, EMB], F32, tag="o", name=f"ot_{qt}")
            nc.vector.tensor_copy(ot[:, 0:512], ps[0][:])
            nc.scalar.activation(ot[:, 512:1024], ps[1][:],
                                 mybir.ActivationFunctionType.Copy)
            if STAGE != "noodma":
                nc.gpsimd.dma_start(out[qt * 128:(qt + 1) * 128, :], ot[:])
        return go

    def queue_outproj(qc):
        for qt4 in range(4):
            st = []
            fin.append(out_mm_slice(qc * 4 + qt4, st))
            fin.append(out_drain_slice(qc * 4 + qt4, st))

    # ---- phase 1: K and V projections for every chunk, Q projection for
    # chunk 0 (later chunks' Q is sliced into the attention loop).
    # PE order K0 V0 K1 Q0 V1 K2 V2 K3 V3; DMAs issued in need order across
    # the SP / ACT / Pool queue rings (the DMA engines serialize copies).
    if STAGE == "noxdma":
        xk0 = preslabs["xk_0"]
    else:
        xk0 = xpool.tile([128, KT_E * 512], BF16, tag="slab", name="xk_0")
        nc.scalar.dma_start(xk0[:, 0: KT_E * 256], xkT[0, :, 0: KT_E * 256])
        nc.scalar.dma_start(xk0[:, KT_E * 256:], xkT[0, :, KT_E * 256:])
    xv0 = slab_dma(nc.sync, xvT, 0, "xv_0")
    xk1 = slab_dma(nc.scalar, xkT, 1, "xk_1")
    xq0 = slab_dma(nc.sync, xqT, 0, "xq_0")
    xv1 = slab_dma(nc.scalar, xvT, 1, "xv_1")
    proj_qk(0, wk_sb, xk0, kT_sb, bkc)
    xk2 = slab_dma(nc.sync, xkT, 2, "xk_2")
    proj_v(0, xv0)
    xv2 = slab_dma(nc.scalar, xvT, 2, "xv_2")
    proj_qk(1, wk_sb, xk1, kT_sb, bkc)
    xk3 = slab_dma(nc.sync, xkT, 3, "xk_3")
    proj_qk(0, wq_sb, xq0, qT_sb, bqc)
    xv3 = slab_dma(nc.scalar, xvT, 3, "xv_3")
    proj_v(1, xv1)
    proj_qk(2, wk_sb, xk2, kT_sb, bkc)
    proj_v(2, xv2)
    proj_qk(3, wk_sb, xk3, kT_sb, bkc)
    proj_v(3, xv3)

    # ---- phase 2: attention per query chunk ----
    if STAGE == "ph1only":
        return
    for qc in range(NQC):
        qcq = 0 if STAGE == "nofin" else qc
        if qc + 1 < NQC and STAGE != "nofin":
            xq_next = slab_dma(nc.sync, xqT, qc + 1, f"xq_{qc + 1}")
        for hp in range(2):
            cps = [cpool.tile([128, 512], F32, tag="ctx", name=f"ctx_{qc}_{hp}_{hi}")
                   for hi in range(2)]

            def ctx_mms(e, kt, hp=hp, cps=cps):
                for hi in range(2):
                    h = hp * 2 + hi
                    vcol = kt * (HPC * 65) + h * 65
                    nc.tensor.matmul(
                        cps[hi][0:65, :], v_sb[:, vcol: vcol + 65],
                        e[:, hi * 512: hi * 512 + 512],
                        start=(kt == 0), stop=(kt == NST - 1),
                    )

            prev = None
            for kt in range(NST):
                sct = next_spool(f"sc_{qc}_{hp}_{kt}")
                for hi in range(2):
                    base = 64 * hi
                    blk = hp * S
                    nc.tensor.matmul(
                        sct[:, hi * 512: hi * 512 + 512],
                        kT_sb[base:base + 64, blk + kt * 128: blk + kt * 128 + 128],
                        qT_sb[base:base + 64, blk + qcq * 512: blk + qcq * 512 + 512],
                        start=True, stop=True,
                    )
                e = epool.tile([128, 1024], BF16, tag="e", name=f"e_{qc}_{hp}_{kt}")
                nc.scalar.activation(e[:], sct[:], EXP, scale=0.125)
                if prev is not None:
                    ctx_mms(*prev)
                prev = (e, kt)
                pop()
            # last ctx step + normalization are deferred into the next
            # window so the seam never stalls on the final exp
            st = []
            fin.insert(0, ctx_tail_slice(ctx_mms, prev, hp, cps))
            queue(1, rden_out_slice(qc, hp))
            queue(2, bs_in_slice(qc, hp, 0, st))
            queue(3, bs_in_slice(qc, hp, 1, st))
            queue(4, mul_slice(qc, hp, 0, cps, st))
            queue(5, mul_slice(qc, hp, 1, cps, st))
            if hp == 0 and qc + 1 < NQC and (do_qp or STAGE == "qpseam"):
                if STAGE == "qpseam":
                    ps = next_spool(f"qp_{qc + 1}")
                    for dq in range(2):
                        for step in range(KT_E):
                            nc.tensor.matmul(
                                ps[:, dq * 512: dq * 512 + 512],
                                wq_sb[:, step * DQ + dq * 128: step * DQ + dq * 128 + 128],
                                xq_next[:, step * 512: step * 512 + 512],
                                start=(step == 0), stop=(step == KT_E - 1),
                            )
                    proj_drain(ps, qT_sb, qc + 1, bqc)
                else:
                    queue_qproj(qc + 1, xq_next)
        if do_out:
            queue_outproj(qc)
    while fin:
        pop()


def _build_nc(bench_iters=None):
    from contextlib import ExitStack

    nc = bacc.Bacc("TRN2", target_bir_lowering=False, debug=False, num_devices=NCORES)
    xqT = nc.dram_tensor("xqT", [NQC, 128, KT_E * 512], BF16, kind="ExternalInput").ap()
    xkT = nc.dram_tensor("xkT", [NQC, 128, KT_E * 512], BF16, kind="ExternalInput").ap()
    xvT = nc.dram_tensor("xvT", [NQC, 128, KT_E * 512], BF16, kind="ExternalInput").ap()
    wqT = nc.dram_tensor("wqT", [EMB, DQ], BF16, kind="ExternalInput").ap()
    wkT = nc.dram_tensor("wkT", [EMB, DQ], BF16, kind="ExternalInput").ap()
    wvT = nc.dram_tensor("wvT", [EMB, DQ], BF16, kind="ExternalInput").ap()
    woT = nc.dram_tensor("woT", [DQ, EMB], BF16, kind="ExternalInput").ap()
    bq = nc.dram_tensor("bq", [1, DQ], F32, kind="ExternalInput").ap()
    bk = nc.dram_tensor("bk", [1, DQ], F32, kind="ExternalInput").ap()
    bv = nc.dram_tensor("bv", [1, DQ], BF16, kind="ExternalInput").ap()
    out = nc.dram_tensor("out", [S, EMB], F32, kind="ExternalOutput").ap()
    nrm_scr = nc.dram_tensor("nrm_scr", [NQC * 2, 2, 512], BF16, kind="Internal").ap()

    with ExitStack() as ctx:
        tc = ctx.enter_context(tile.TileContext(nc))
        _mha(ctx, tc, xqT, xkT, xvT, wqT, wkT, wvT, woT, bq, bk, bv, out,
             nrm_scr=nrm_scr, bench_iters=bench_iters)
    nc.compile()
    return nc


def _chunk_major(x):
    """[S, EMB] fp32 -> bf16 x.T as [NQC, 128, KT_E*512] (slab per chunk)."""
    xt = np.asarray(x, np.float32).T.astype(NP_BF16)  # [EMB, S]
    return np.ascontiguousarray(
        xt.reshape(KT_E, 128, NQC, 512).transpose(2, 1, 0, 3).reshape(NQC, 128, KT_E * 512)
    )


def make_in_maps(query, key, value, Wq, bq, Wk, bk, Wv, bv, Wo, bo):
    in_maps = []
    for c in range(NCORES):
        b, g = divmod(c, 4)
        rows = slice(g * DQ, (g + 1) * DQ)
        in_maps.append({
            "xqT": _chunk_major(query[b]),
            "xkT": _chunk_major(key[b]),
            "xvT": _chunk_major(value[b]),
            "wqT": np.ascontiguousarray(np.asarray(Wq[rows].T, np.float32).astype(NP_BF16)),
            "wkT": np.ascontiguousarray(np.asarray(Wk[rows].T, np.float32).astype(NP_BF16)),
            "wvT": np.ascontiguousarray(np.asarray(Wv[rows].T, np.float32).astype(NP_BF16)),
            "woT": np.ascontiguousarray(np.asarray(Wo[:, rows].T, np.float32).astype(NP_BF16)),
            "bq": np.ascontiguousarray(np.asarray(bq[rows], np.float32)[None, :]),
            "bk": np.ascontiguousarray(np.asarray(bk[rows], np.float32)[None, :]),
            "bv": np.ascontiguousarray(np.asarray(bv[rows], np.float32).astype(NP_BF16)[None, :]),
        })
    return in_maps


def kernel(query, key, value, Wq, bq, Wk, bk, Wv, bv, Wo, bo):
    global _NC, LAST_RESULT
    if _NC is None:
        _NC = _build_nc()

    in_maps = make_in_maps(query, key, value, Wq, bq, Wk, bk, Wv, bv, Wo, bo)
    res = bass_utils.run_bass_kernel_spmd(
        _NC, in_maps, core_ids=list(range(NCORES)), trace=TRACE
    )
    LAST_RESULT = res

    out = np.zeros((B, S, EMB), np.float32)
    for c in range(NCORES):
        out[c // 4] += res.results[c]["out"]
    out += np.asarray(bo, np.float32)[None, None, :]
    return out


# revision 61
# speedup vs baseline: 1.0732x; 1.0732x over previous
"""Multi-head attention (B=2, S=2048, EMB=1024, 16 heads) on 8 Trainium2 cores.

Sharding: core c -> batch c//4, head-group c%4 (4 heads = 256 projection dims).
Each core computes Q/K projections in transposed layout (head-dim on
partitions), V natural, attention without max subtraction (scores ~ N(0,1) in
fp32), the softmax denominator via a ones-column appended to V (free inside
the ctx matmul M=65), and a row-parallel partial of the output projection.
The host sums the 4 partials per batch and adds the output bias.

v2 design (driven by the TimelineSim cost model, which tracks HW within ~6%):
  - matmul cost = out free-size x cycles/col, independent of K -> ctx matmuls
    use single K=128 chains (16 kt steps) instead of K=64 half-chains,
    halving ctx PE time. Consecutive mms alternate PSUM banks (hi index) so
    LDWEIGHTS pipelines on HW.
  - exp runs on [128, 1024] tiles (both heads of a pair share one 2-bank
    PSUM score tile) halving the per-instruction ACT access overhead.
  - all SBUF/DRAM tensors are bf16 (PSUM stays fp32): same PE rate, half
    the DMA traffic (phase 1 was DMA-bound), half the SBUF footprint.
  - x inputs arrive as one [128, 4096] slab DMA per (tensor, chunk) spread
    over the SP/ACT/DVE queue rings (96 small DMAs -> 12 big ones).
  - softmax 1/den broadcast: one K=2 matmul per head-pair fans both heads'
    reciprocal rows across 128 partitions (sel2 one-hot).
  - normalization is deferred by one head-pair, out-projection by one query
    chunk; both are sliced into the ACT-bound attention loop (one slice per
    kt step) where PE/DVE have slack.
PSUM: spoolA/B 1x[128,1024] each (scores / projections / bcast rotating,
4 banks) + ctx pool 4x[128,512] (ctx chains + out-proj, 4 banks) = 8 banks.
"""

import numpy as np

import concourse.tile as tile
from concourse import bacc, mybir
from concourse import bass_utils

EMB = 1024
S = 2048
B = 2
HPC = 4            # heads per core
DQ = HPC * 64      # 256 projection dims per core
NCORES = 8

F32 = mybir.dt.float32
BF16 = mybir.dt.bfloat16
EXP = mybir.ActivationFunctionType.Exp

KT_E = EMB // 128  # 8 contraction tiles over EMB
NQC = S // 512     # 4 query chunks
NST = S // 128     # 16 sequence tiles

NP_BF16 = mybir.dt.np(BF16)

_NC = None
TRACE = False
LAST_RESULT = None
STAGE = "full"   # "full" | "nofin" (skip norm/outproj/qproj) | "noodma" (skip out DMA) | "noxdma" (reuse one slab)


def _mha(ctx, tc, xqT, xkT, xvT, wqT, wkT, wvT, woT, bq, bk, bv, out,
         bench_iters=None):
    nc = tc.nc

    cstp = ctx.enter_context(tc.tile_pool(name="const", bufs=1))
    xpool = ctx.enter_context(tc.tile_pool(name="xin", bufs=8))
    epool = ctx.enter_context(tc.tile_pool(name="exp", bufs=4))
    opool = ctx.enter_context(tc.tile_pool(name="osb", bufs=3))
    spoolA = ctx.enter_context(tc.tile_pool(name="scpsA", bufs=1, space="PSUM"))
    spoolB = ctx.enter_context(tc.tile_pool(name="scpsB", bufs=1, space="PSUM"))
    cpool = ctx.enter_context(tc.tile_pool(name="ctxps", bufs=4, space="PSUM"))
    bspool = ctx.enter_context(tc.tile_pool(name="bsnorm", bufs=2))
    nrmpool = ctx.enter_context(tc.tile_pool(name="nrmscr", bufs=2, space="DRAM"))

    # ---- persistent SBUF tensors ----
    ones_row = cstp.tile([1, 512], BF16)
    # 1/den rows live at partitions 0 and 64 (legal engine starts); they are
    # broadcast across partitions via a DRAM round-trip DMA (stride-0 source),
    # keeping the whole softmax normalization off the PE instruction queue.
    rdens = [cstp.tile([65, 512], BF16, name=f"rden{i}") for i in range(2)]
    with nc.allow_low_precision(reason="bf16 constants"):
        nc.vector.memset(ones_row[:], 1.0)

    wq_sb = cstp.tile([128, KT_E * DQ], BF16)   # [128, 2048]: wq_sb[p, n*256+m] = WqT[n*128+p, m]
    wk_sb = cstp.tile([128, KT_E * DQ], BF16)
    wv_sb = cstp.tile([128, KT_E * DQ], BF16)
    wo_sb = cstp.tile([128, 2 * EMB], BF16)     # wo_sb[p, n*1024+f] = WoT[n*128+p, f]
    # per-partition bias columns: b?c[p, dq] = bias[dq*128+p]
    bqc = cstp.tile([128, 2], F32)
    bkc = cstp.tile([128, 2], F32)
    bv_sb = cstp.tile([1, DQ], BF16)

    preslabs = {}
    if STAGE == "noxdma":
        for nm, src, qc in ([(f"xk_{i}", xkT, i) for i in range(NQC)]
                            + [(f"xv_{i}", xvT, i) for i in range(NQC)]
                            + [(f"xq_{i}", xqT, i) for i in range(NQC)]):
            t = cstp.tile([128, KT_E * 512], BF16, name=f"pre_{nm}")
            nc.sync.dma_start(t[:], src[qc])
            preslabs[nm] = t

    # results of phase 1 kept resident
    kT_sb = cstp.tile([128, 2 * S], BF16)       # [dq-block 2][s 2048]
    qT_sb = cstp.tile([128, 2 * S], BF16)
    ctxT_sb = cstp.tile([128, 2 * S], BF16)
    v_sb = cstp.tile([128, NST * (HPC * 65)], BF16)  # per s-tile: 4 heads x (64 V + ones col)
    with nc.allow_low_precision(reason="init"):
        nc.vector.memset(ctxT_sb[:], 0.001)
        nc.vector.memset(qT_sb[:], 0.001)
    with nc.allow_low_precision(reason="bf16 ones column"):
        nc.vector.memset(
            v_sb[:].rearrange("p (t h m) -> p t h m", t=NST, h=HPC)[:, :, :, 64:65],
            1.0,
        )

    # ---- weight / bias DMAs; wk leads the SP ring so the first projection
    # starts ASAP. Slow SWDGE (gpsimd) ring only carries non-urgent copies. ----
    nc.sync.dma_start(
        wk_sb[:].rearrange("p (n m) -> p n m", n=KT_E),
        wkT.rearrange("(n p) m -> p n m", p=128),
    )
    nc.scalar.dma_start(
        wv_sb[:].rearrange("p (n m) -> p n m", n=KT_E),
        wvT.rearrange("(n p) m -> p n m", p=128),
    )
    nc.gpsimd.dma_start(bkc[:], bk.rearrange("o (d p) -> p (o d)", p=128))
    nc.gpsimd.dma_start(bqc[:], bq.rearrange("o (d p) -> p (o d)", p=128))
    nc.gpsimd.dma_start(bv_sb[:], bv[:])
    nc.gpsimd.dma_start(
        wq_sb[:].rearrange("p (n m) -> p n m", n=KT_E),
        wqT.rearrange("(n p) m -> p n m", p=128),
    )
    nc.gpsimd.dma_start(
        wo_sb[:].rearrange("p (n m) -> p n m", n=2),
        woT.rearrange("(n p) m -> p n m", p=128),
    )

    def body():
        _body(tc, nc, xqT, xkT, xvT, out, ones_row, nrmpool, rdens, wq_sb, wk_sb,
              wv_sb, wo_sb, bqc, bkc, bv_sb, kT_sb, qT_sb, ctxT_sb, v_sb,
              xpool, epool, opool, spoolA, spoolB, cpool, bspool, preslabs)

    if bench_iters:
        hints = (
            mybir.EngineType.PE,
            mybir.EngineType.Activation,
            mybir.EngineType.DVE,
            mybir.EngineType.SP,
            mybir.EngineType.Pool,
        )
        with tc.For_i(0, bench_iters, 1, hint_engines=hints):
            body()
    else:
        body()


def _body(tc, nc, xqT, xkT, xvT, out, ones_row, nrmpool, rdens, wq_sb, wk_sb,
          wv_sb, wo_sb, bqc, bkc, bv_sb, kT_sb, qT_sb, ctxT_sb, v_sb,
          xpool, epool, opool, spoolA, spoolB, cpool, bspool, preslabs):
    fin = []  # FIFO of deferred finish slices, popped one per kt step

    def pop():
        if fin:
            fin.pop(0)()

    do_norm = STAGE not in ("nofin", "nonorm", "on_out", "on_qp")
    do_out = STAGE not in ("nofin", "noout", "on_norm", "on_qp", "noodma2")
    do_qp = STAGE not in ("nofin", "noqp", "on_norm", "on_out", "qpseam")

    def queue(idx, sl):
        if do_norm:
            fin.insert(idx, sl) if idx is not None else fin.append(sl)

    spools = [spoolA, spoolB]
    sp_state = [0]

    def next_spool(name):
        t = spools[sp_state[0] % 2].tile([128, 1024], F32, tag="sc", name=name)
        sp_state[0] += 1
        return t

    def slab_dma(eng, src, qc, name):
        if STAGE == "noxdma":
            return preslabs[name]
        t = xpool.tile([128, KT_E * 512], BF16, tag="slab", name=name)
        eng.dma_start(t[:], src[qc])
        return t

    def proj_mms(ps, qc, w_sb, xs, kts):
        # two dq chains into the two banks of one [128,1024] psum tile
        for kt in kts:
            for dq in range(2):
                nc.tensor.matmul(
                    ps[:, dq * 512: dq * 512 + 512],
                    w_sb[:, kt * DQ + dq * 128: kt * DQ + dq * 128 + 128],
                    xs[:, kt * 512: kt * 512 + 512],
                    start=(kt == 0), stop=(kt == KT_E - 1),
                )

    def proj_drain(ps, dst_sb, qc, bcol):
        for dq in range(2):
            nc.vector.tensor_scalar_add(
                dst_sb[:, dq * S + qc * 512: dq * S + qc * 512 + 512],
                ps[:, dq * 512: dq * 512 + 512], bcol[:, dq: dq + 1],
            )

    def proj_qk(qc, w_sb, xs, dst_sb, bcol):
        ps = next_spool(f"p_{qc}")
        proj_mms(ps, qc, w_sb, xs, range(KT_E))
        proj_drain(ps, dst_sb, qc, bcol)

    def proj_v(qc, xv):
        for pair in range(2):
            vt = next_spool(f"vps_{qc}_{pair}")
            for kt in range(KT_E):
                for i in range(2):
                    sti = pair * 2 + i
                    nc.tensor.matmul(
                        vt[:, i * 512: i * 512 + DQ],
                        xv[:, kt * 512 + sti * 128: kt * 512 + sti * 128 + 128],
                        wv_sb[:, kt * DQ: kt * DQ + DQ],
                        start=(kt == 0), stop=False,
                    )
            for i in range(2):
                nc.tensor.matmul(
                    vt[:, i * 512: i * 512 + DQ], ones_row[0:1, 0:128], bv_sb[0:1, :],
                    start=False, stop=True,
                )
            for i in range(2):
                st = qc * 4 + pair * 2 + i
                dst = v_sb[:, st * (HPC * 65): (st + 1) * (HPC * 65)]
                nc.vector.tensor_copy(
                    dst.rearrange("p (h m) -> p h m", h=HPC)[:, :, 0:64],
                    vt[:, i * 512: i * 512 + DQ].rearrange("p (h m) -> p h m", h=HPC),
                )

    # ---- finish slices -------------------------------------------------
    # Normalization of (qc, hp): 1/den rows (DVE) -> PE broadcast of each
    # head's 1/den into the spare rows 64:128 of the *other* head's ctx tile
    # (no extra PSUM bank) -> per-head multiply into ctxT_sb.
    def ctx_tail_slice(ctx_mms, prev, qc, hp, cps, st):
        def go():
            ctx_mms(*prev)
            for hi in range(2):
                with nc.allow_low_precision(reason="bf16 reciprocal of softmax denom"):
                    nc.vector.reciprocal(
                        rdens[hp][64 * hi: 64 * hi + 1, :], cps[hi][64:65, :]
                    )
            scr = nrmpool.tile([2, 512], BF16, tag="scr", name=f"scr_{qc}_{hp}")
            st.append(scr)
            r = rdens[hp][:]
            src_ap = AP(tensor=r.tensor, offset=r.offset, ap=[[64 * 512, 2], [1, 512]])
            nc.sync.dma_start(scr[:], src_ap)
        return go

    from concourse.bass import AP

    def bs_in_slice(qc, hp, hi, st):
        def go():
            scr = st[0]
            if hi == 0:
                st.append(bspool.tile([128, 512], BF16, tag="bs", name=f"bs_{qc}_{hp}"))
            bs = st[1]
            s = scr[hi: hi + 1, :]
            src_ap = AP(tensor=s.tensor, offset=s.offset, ap=[[0, 64], [1, 512]])
            nc.sync.dma_start(bs[64 * hi: 64 * hi + 64, :], src_ap)
        return go

    def mul_slice(qc, hp, hi, cps, st):
        def go():
            bs = st[1]
            nc.vector.tensor_mul(
                ctxT_sb[64 * hi: 64 * hi + 64, hp * S + qc * 512: hp * S + qc * 512 + 512],
                cps[hi][0:64, :],
                bs[64 * hi: 64 * hi + 64, :],
            )
        return go

    def out_mm_slice(qt, st):
        def go():
            ps = [cpool.tile([128, 512], F32, tag="ctx", name=f"ops_{qt}_{fc}")
                  for fc in range(2)]
            for dq in range(2):
                for fc in range(2):
                    nc.tensor.matmul(
                        ps[fc][:],
                        ctxT_sb[:, dq * S + qt * 128: dq * S + qt * 128 + 128],
                        wo_sb[:, dq * EMB + fc * 512: dq * EMB + fc * 512 + 512],
                        start=(dq == 0), stop=(dq == 1),
                    )
            st.append(ps)
        return go

    def out_drain_slice(qt, st):
        def go():
            ps = st.pop(0)
            ot = opool.tile([128, EMB], F32, tag="o", name=f"ot_{qt}")
            nc.vector.tensor_copy(ot[:, 0:512], ps[0][:])
            nc.scalar.activation(ot[:, 512:1024], ps[1][:],
                                 mybir.ActivationFunctionType.Copy)
            if STAGE != "noodma":
                nc.gpsimd.dma_start(out[qt * 128:(qt + 1) * 128, :], ot[:])
        return go

    def queue_outproj(qc):
        fin.append(lambda: None)
        for qt4 in range(4):
            st = []
            fin.append(out_mm_slice(qc * 4 + qt4, st))
            fin.append(out_drain_slice(qc * 4 + qt4, st))

    # ---- phase 1: K and V projections for every chunk, Q projection for
    # chunk 0 (later chunks' Q is sliced into the attention loop).
    # PE order K0 V0 K1 Q0 V1 K2 V2 K3 V3; DMAs issued in need order across
    # the SP / ACT / Pool queue rings (the DMA engines serialize copies).
    if STAGE == "noxdma":
        xk0 = preslabs["xk_0"]
    else:
        xk0 = xpool.tile([128, KT_E * 512], BF16, tag="slab", name="xk_0")
        nc.scalar.dma_start(xk0[:, 0: KT_E * 256], xkT[0, :, 0: KT_E * 256])
        nc.scalar.dma_start(xk0[:, KT_E * 256:], xkT[0, :, KT_E * 256:])
    xv0 = slab_dma(nc.sync, xvT, 0, "xv_0")
    xq0 = slab_dma(nc.scalar, xqT, 0, "xq_0")
    xk1 = slab_dma(nc.sync, xkT, 1, "xk_1")
    xv1 = slab_dma(nc.scalar, xvT, 1, "xv_1")
    proj_qk(0, wk_sb, xk0, kT_sb, bkc)
    xq1 = slab_dma(nc.sync, xqT, 1, "xq_1")
    proj_v(0, xv0)
    xk2 = slab_dma(nc.scalar, xkT, 2, "xk_2")
    proj_qk(0, wq_sb, xq0, qT_sb, bqc)
    xv2 = slab_dma(nc.sync, xvT, 2, "xv_2")
    proj_qk(1, wk_sb, xk1, kT_sb, bkc)
    xq2 = slab_dma(nc.scalar, xqT, 2, "xq_2")
    proj_v(1, xv1)
    xk3 = slab_dma(nc.sync, xkT, 3, "xk_3")
    proj_qk(1, wq_sb, xq1, qT_sb, bqc)
    xv3 = slab_dma(nc.scalar, xvT, 3, "xv_3")
    proj_qk(2, wk_sb, xk2, kT_sb, bkc)
    xq3 = slab_dma(nc.sync, xqT, 3, "xq_3")
    proj_v(2, xv2)
    proj_qk(2, wq_sb, xq2, qT_sb, bqc)
    proj_qk(3, wk_sb, xk3, kT_sb, bkc)
    proj_v(3, xv3)
    proj_qk(3, wq_sb, xq3, qT_sb, bqc)

    # ---- phase 2: attention per query chunk ----
    if STAGE == "ph1only":
        return
    for qc in range(NQC):
        qcq = qc
        for hp in range(2):
            cps = [cpool.tile([128, 512], F32, tag="ctx", name=f"ctx_{qc}_{hp}_{hi}")
                   for hi in range(2)]

            def ctx_mms(e, kt, hp=hp, cps=cps):
                for hi in range(2):
                    h = hp * 2 + hi
                    vcol = kt * (HPC * 65) + h * 65
                    nc.tensor.matmul(
                        cps[hi][0:65, :], v_sb[:, vcol: vcol + 65],
                        e[:, hi * 512: hi * 512 + 512],
                        start=(kt == 0), stop=(kt == NST - 1),
                    )

            prev = None
            for kt in range(NST):
                sct = next_spool(f"sc_{qc}_{hp}_{kt}")
                for hi in range(2):
                    base = 64 * hi
                    blk = hp * S
                    nc.tensor.matmul(
                        sct[:, hi * 512: hi * 512 + 512],
                        kT_sb[base:base + 64, blk + kt * 128: blk + kt * 128 + 128],
                        qT_sb[base:base + 64, blk + qcq * 512: blk + qcq * 512 + 512],
                        start=True, stop=True,
                    )
                e = epool.tile([128, 1024], BF16, tag="e", name=f"e_{qc}_{hp}_{kt}")
                nc.scalar.activation(e[:], sct[:], EXP, scale=0.125)
                if prev is not None:
                    ctx_mms(*prev)
                prev = (e, kt)
                pop()
            # last ctx step + normalization are deferred into the next
            # window so the seam never stalls on the final exp
            st = []
            fin.insert(0, ctx_tail_slice(ctx_mms, prev, qc, hp, cps, st))
            queue(1, bs_in_slice(qc, hp, 0, st))
            queue(2, bs_in_slice(qc, hp, 1, st))
            queue(4, mul_slice(qc, hp, 0, cps, st))
            queue(5, mul_slice(qc, hp, 1, cps, st))
        if do_out:
            queue_outproj(qc)
    while fin:
        pop()


def _build_nc(bench_iters=None):
    from contextlib import ExitStack

    nc = bacc.Bacc("TRN2", target_bir_lowering=False, debug=False, num_devices=NCORES)
    xqT = nc.dram_tensor("xqT", [NQC, 128, KT_E * 512], BF16, kind="ExternalInput").ap()
    xkT = nc.dram_tensor("xkT", [NQC, 128, KT_E * 512], BF16, kind="ExternalInput").ap()
    xvT = nc.dram_tensor("xvT", [NQC, 128, KT_E * 512], BF16, kind="ExternalInput").ap()
    wqT = nc.dram_tensor("wqT", [EMB, DQ], BF16, kind="ExternalInput").ap()
    wkT = nc.dram_tensor("wkT", [EMB, DQ], BF16, kind="ExternalInput").ap()
    wvT = nc.dram_tensor("wvT", [EMB, DQ], BF16, kind="ExternalInput").ap()
    woT = nc.dram_tensor("woT", [DQ, EMB], BF16, kind="ExternalInput").ap()
    bq = nc.dram_tensor("bq", [1, DQ], F32, kind="ExternalInput").ap()
    bk = nc.dram_tensor("bk", [1, DQ], F32, kind="ExternalInput").ap()
    bv = nc.dram_tensor("bv", [1, DQ], BF16, kind="ExternalInput").ap()
    out = nc.dram_tensor("out", [S, EMB], F32, kind="ExternalOutput").ap()

    with ExitStack() as ctx:
        tc = ctx.enter_context(tile.TileContext(nc))
        _mha(ctx, tc, xqT, xkT, xvT, wqT, wkT, wvT, woT, bq, bk, bv, out,
             bench_iters=bench_iters)
    nc.compile()
    return nc


def _chunk_major(x):
    """[S, EMB] fp32 -> bf16 x.T as [NQC, 128, KT_E*512] (slab per chunk)."""
    xt = np.asarray(x, np.float32).T.astype(NP_BF16)  # [EMB, S]
    return np.ascontiguousarray(
        xt.reshape(KT_E, 128, NQC, 512).transpose(2, 1, 0, 3).reshape(NQC, 128, KT_E * 512)
    )


def make_in_maps(query, key, value, Wq, bq, Wk, bk, Wv, bv, Wo, bo):
    in_maps = []
    for c in range(NCORES):
        b, g = divmod(c, 4)
        rows = slice(g * DQ, (g + 1) * DQ)
        in_maps.append({
            "xqT": _chunk_major(query[b]),
            "xkT": _chunk_major(key[b]),
            "xvT": _chunk_major(value[b]),
            "wqT": np.ascontiguousarray(np.asarray(Wq[rows].T, np.float32).astype(NP_BF16)),
            "wkT": np.ascontiguousarray(np.asarray(Wk[rows].T, np.float32).astype(NP_BF16)),
            "wvT": np.ascontiguousarray(np.asarray(Wv[rows].T, np.float32).astype(NP_BF16)),
            "woT": np.ascontiguousarray(np.asarray(Wo[:, rows].T, np.float32).astype(NP_BF16)),
            "bq": np.ascontiguousarray(np.asarray(bq[rows], np.float32)[None, :]),
            "bk": np.ascontiguousarray(np.asarray(bk[rows], np.float32)[None, :]),
            "bv": np.ascontiguousarray(np.asarray(bv[rows], np.float32).astype(NP_BF16)[None, :]),
        })
    return in_maps


def kernel(query, key, value, Wq, bq, Wk, bk, Wv, bv, Wo, bo):
    global _NC, LAST_RESULT
    if _NC is None:
        _NC = _build_nc()

    in_maps = make_in_maps(query, key, value, Wq, bq, Wk, bk, Wv, bv, Wo, bo)
    res = bass_utils.run_bass_kernel_spmd(
        _NC, in_maps, core_ids=list(range(NCORES)), trace=TRACE
    )
    LAST_RESULT = res

    out = np.zeros((B, S, EMB), np.float32)
    for c in range(NCORES):
        out[c // 4] += res.results[c]["out"]
    out += np.asarray(bo, np.float32)[None, None, :]
    return out
